# revision 9
# baseline (speedup 1.0000x reference)
"""Trainium2 Bass kernel for nn_Attention_24043226923261.

Per-pixel cross-attention: RMSNorm(c) -> kv proj -> softmax over N=8 context
slices with a query shared across the 32x32 spatial grid -> out proj.

Sharding: data-parallel over B=8 across the 8 NeuronCores (core b owns batch
b). Zero collectives.

Host-side weight folding (exact math, same as the 100us baseline):
  - query path qh = silu(emb[q]@w1+b1)@w2+b2 is tiny ([8,512]); dots =
    c_norm @ (w_k @ qh^T), so qh, attn_scale and rms_w fold into a per-core
    [256,8] matrix wq.  k is never materialized.
  - rms_w folds into wv/wq; the per-token rsqrt(mean(c^2)) scale s[t,n] is
    applied on device (k-side inside the softmax logits, v-side folded into
    the softmax weights).
  - out proj computed transposed (out^T = wo^T @ h^T) so the result lands
    channel-major [256, H*W] = the required output layout.

v2 redesign (from trace analysis of the 100.7us baseline):
  - Baseline bottleneck was DVE at ~70% busy; 42us of that was the 64
    combine multiplies h += a*(v from PSUM f32) at 659ns each, plus the
    first ~10us were pure DMA and pass-0 was latency-bound.
  - The ACT engine (36% busy) now copies v tiles PSUM->SBUF as bf16; the
    hs axis is reordered to (d,e) so the per-(e)-broadcast multiply keeps a
    packed stride-1 last dim -> DVE 2x_1p mode (327ns vs 659ns).
  - Combine adds split by whole token-tiles to GPSIMD (no cross-engine
    ping-pong inside one accumulation chain).
  - c loads spread over 4 DMA queues (sync/scalar/gpsimd/tensor); squares
    split DVE/ACT; pass-0 (tiny shingled dots/mean matmuls) overlaps the
    v-matmul pipeline.
  - Output written bf16 (rel-err budget allows it), halving out-DMA.
"""

import sys

for _p in ("/opt/trn_rl_repo",):
    if _p not in sys.path:
        sys.path.insert(0, _p)

import numpy as np


B = 8
N = 8          # context slices (softmax axis)
CH = 256       # channels / hidden
H = W = 32
T = H * W      # 1024 spatial tokens per batch
HEADS = 8
HD = 64        # head dim
HS = HEADS * HD  # 512
EPS = 1e-6
NCORES = 8
PT = 128       # partition tile
TT = T // PT   # 8 token tiles
KCH = CH // PT  # 2 contraction chunks over channels
KHS = HS // PT  # 4 contraction chunks over (d, e)
GRP = 4        # token tiles per out-proj batch

# token tiles whose combine ADD chains run wholly on GPSIMD
GPS_TTS = (0, 1, 4)
# token tiles whose v is multiplied straight from PSUM f32 (no ACT copy)
PSUM_TTS = ()


def _kernel_body(nc, tc, d):
    from contextlib import ExitStack

    from concourse import mybir

    AF = mybir.ActivationFunctionType
    ALU = mybir.AluOpType
    AX = mybir.AxisListType
    f32 = mybir.dt.float32
    bf16 = mybir.dt.bfloat16

    with ExitStack() as ctx:
        const = ctx.enter_context(tc.tile_pool(name="const", bufs=1))
        cpool = ctx.enter_context(tc.tile_pool(name="c", bufs=1))
        csqp = ctx.enter_context(tc.tile_pool(name="csq", bufs=3))
        sp = ctx.enter_context(tc.tile_pool(name="s", bufs=1))
        ep = ctx.enter_context(tc.tile_pool(name="e", bufs=1))
        vsb = ctx.enter_context(tc.tile_pool(name="vsb", bufs=1))
        hp = ctx.enter_context(tc.tile_pool(name="h", bufs=1))
        prodp = ctx.enter_context(tc.tile_pool(name="prod", bufs=3))
        htp = ctx.enter_context(tc.tile_pool(name="ht", bufs=2))
        outp = ctx.enter_context(tc.tile_pool(name="o", bufs=2))
        psD = ctx.enter_context(tc.tile_pool(name="psD", bufs=1, space="PSUM"))
        psM = ctx.enter_context(tc.tile_pool(name="psM", bufs=1, space="PSUM"))
        psV = ctx.enter_context(tc.tile_pool(name="psV", bufs=3, space="PSUM"))
        psT = ctx.enter_context(tc.tile_pool(name="psT", bufs=2, space="PSUM"))
        psO = ctx.enter_context(tc.tile_pool(name="psO", bufs=1, space="PSUM"))

        # ---- eps for the sqrt bias (DVE, before its DMA issues) ----
        eps_sb = const.tile([PT, 1], f32, tag="eps", name="eps")
        nc.vector.memset(eps_sb[:], EPS)

        # ---- DMA issues, spread over 4 queues ----
        # c[n] is ONE [128, 2*T] tile per k handled as one [PT, KCH*T]? No:
        # keep per (n) a single [PT, KCH*T] tile holding k0|k1 so the square
        # is one DVE op; the two k chunks are separate DMAs into halves.
        c_sb = {}

        def _load_c(eng, n):
            t = cpool.tile([PT, KCH * T], bf16, tag=f"c{n}", name=f"c{n}")
            for k in range(KCH):
                eng.dma_start(t[:, k * T:(k + 1) * T],
                              d["c"][n, k * PT:(k + 1) * PT, :])
            c_sb[n] = t

        # sync: c0, c1 first (gate everything), then c6 + pass-1 weights
        _load_c(nc.sync, 0)
        _load_c(nc.sync, 1)
        wv_sb = []
        for k in range(KCH):
            t = const.tile([PT, HS], bf16, tag=f"wv{k}", name=f"wv{k}")
            nc.sync.dma_start(t[:], d["wv"][k * PT:(k + 1) * PT, :])
            wv_sb.append(t)
        _load_c(nc.sync, 6)
        wo_sb = []
        for k in range(KHS):
            t = const.tile([PT, CH], bf16, tag=f"wo{k}", name=f"wo{k}")
            nc.sync.dma_start(t[:], d["wo"][k * PT:(k + 1) * PT, :])
            wo_sb.append(t)
        bo_sb = []
        for m in range(CH // PT):
            t = const.tile([PT, 1], f32, tag=f"bo{m}", name=f"bo{m}")
            nc.sync.dma_start(t[:], d["bo"][m * PT:(m + 1) * PT, :])
            bo_sb.append(t)
        eye_sb = const.tile([PT, PT], bf16, tag="eye", name="eye")
        nc.sync.dma_start(eye_sb[:], d["eye"][:, :])

        # scalar queue: wq + invc first (needed by first dots ~2.6us),
        # then c2, c3 (ACT squares them itself)
        wq_sb = []
        invc_sb = []
        for k in range(KCH):
            t = const.tile([PT, HEADS], bf16, tag=f"wq{k}", name=f"wq{k}")
            nc.scalar.dma_start(t[:], d["wq"][k * PT:(k + 1) * PT, :])
            wq_sb.append(t)
        for k in range(KCH):
            t = const.tile([PT, 1], bf16, tag=f"invc{k}", name=f"invc{k}")
            nc.scalar.dma_start(t[:], d["invc"][k * PT:(k + 1) * PT, :])
            invc_sb.append(t)
        _load_c(nc.scalar, 2)
        _load_c(nc.scalar, 3)
        # gpsimd queue: c4, c5, c7
        _load_c(nc.gpsimd, 4)
        _load_c(nc.gpsimd, 5)
        _load_c(nc.gpsimd, 7)

        # ---- pass-0 elementwise: squares (DVE mostly, ACT for n=2,3) ----
        csq = {}
        for n in (0, 1):
            t = csqp.tile([PT, KCH * T], bf16, tag="csq", name=f"csq{n}")
            nc.vector.tensor_mul(t[:], c_sb[n][:], c_sb[n][:])
            csq[n] = t
        for n in (2, 3):
            t = csqp.tile([PT, KCH * T], bf16, tag="csqa", name=f"csq{n}")
            nc.scalar.activation(t[:], c_sb[n][:], AF.Square)
            csq[n] = t
        for n in (4, 5, 6, 7):
            t = csqp.tile([PT, KCH * T], bf16, tag="csq", name=f"csq{n}")
            nc.vector.tensor_mul(t[:], c_sb[n][:], c_sb[n][:])
            csq[n] = t

        # ---- pass-0 PE: dots + mean per n (tiny shingled matmuls) ----
        # D_ps cols = (tt, n, e); mean_ps cols = (tt, n)
        D_ps = psD.tile([PT, TT * N * HEADS], f32, name="D")
        Dv = D_ps[:].rearrange("p (a n e) -> p a n e", a=TT, n=N)
        mean_ps = psM.tile([PT, TT * N], f32, name="mean")
        mv = mean_ps[:].rearrange("p (a n) -> p a n", n=N)
        for n in range(N):
            for tt in range(TT):
                for k in range(KCH):
                    nc.tensor.matmul(
                        Dv[:, tt, n, :],
                        c_sb[n][:, k * T + tt * PT: k * T + (tt + 1) * PT],
                        wq_sb[k][:],
                        start=(k == 0), stop=(k == KCH - 1),
                    )
            for tt in range(TT):
                for k in range(KCH):
                    nc.tensor.matmul(
                        mv[:, tt, n: n + 1],
                        csq[n][:, k * T + tt * PT: k * T + (tt + 1) * PT],
                        invc_sb[k][:],
                        start=(k == 0), stop=(k == KCH - 1),
                    )

        # ---- s = 1/sqrt(mean + eps): one ACT sqrt + one DVE reciprocal ----
        sq_all = sp.tile([PT, TT * N], f32, tag="sq", name="sq_all")
        nc.scalar.activation(sq_all[:], mean_ps[:], AF.Sqrt, bias=eps_sb[:])
        s_all = sp.tile([PT, TT * N], f32, tag="s", name="s_all")
        nc.vector.reciprocal(s_all[:], sq_all[:])
        # broadcast views over e (innermost), layout (tt, n, e)
        s_bc = s_all[:].rearrange("p (a n o) -> p a n o", n=N, o=1) \
                       .broadcast_to([PT, TT, N, HEADS])

        # ---- softmax (full-width single ops) ----
        Dsc = ep.tile([PT, TT * N * HEADS], bf16, tag="Dsc", name="Dsc")
        nc.vector.tensor_mul(
            Dsc[:].rearrange("p (a n e) -> p a n e", a=TT, n=N), Dv, s_bc)
        E = ep.tile([PT, TT * N * HEADS], bf16, tag="E", name="E")
        nc.scalar.activation(E[:], Dsc[:], AF.Exp)
        # Z over n (strided view puts n innermost)
        Z = ep.tile([PT, TT * HEADS], f32, tag="Z", name="Z")
        nc.vector.tensor_reduce(
            Z[:], E[:].rearrange("p (a n e) -> p a e n", a=TT, n=N),
            axis=AX.X, op=ALU.add)
        rZ = ep.tile([PT, TT * HEADS], bf16, tag="rZ", name="rZ")
        with nc.allow_low_precision(reason="softmax weights are bf16 anyway"):
            nc.vector.reciprocal(rZ[:], Z[:])
        # av = E * rZ_bc(over n) * s_bc(over e); rZ-mul is 2x (bf16, packed e)
        rZ_bc = rZ[:].rearrange("p (a o e) -> p a o e", o=1, e=HEADS) \
                     .broadcast_to([PT, TT, N, HEADS])
        av1 = ep.tile([PT, TT * N * HEADS], bf16, tag="av1", name="av1")
        av1v = av1[:].rearrange("p (a n e) -> p a n e", a=TT, n=N)
        nc.vector.tensor_mul(
            av1v, E[:].rearrange("p (a n e) -> p a n e", a=TT, n=N), rZ_bc)
        av = ep.tile([PT, TT * N * HEADS], bf16, tag="av", name="av")
        avv = av[:].rearrange("p (a n o e) -> p a n o e", a=TT, n=N, o=1)
        nc.vector.tensor_mul(
            av[:].rearrange("p (a n e) -> p a n e", a=TT, n=N), av1v, s_bc)

        # ---- v matmuls + ACT psum->sbuf bf16 copies ----
        # v cols are (d, e): wv was column-permuted on host.
        v_ps = {}
        v_sb = {}

        def _emit_v(tt, n):
            ps = psV.tile([PT, HS], f32, tag="v", name=f"v{tt}_{n}")
            for k in range(KCH):
                nc.tensor.matmul(
                    ps[:],
                    c_sb[n][:, k * T + tt * PT: k * T + (tt + 1) * PT],
                    wv_sb[k][:],
                    start=(k == 0), stop=(k == KCH - 1),
                )
            v_ps[tt, n] = ps

        def _emit_vcopy(tt, n):
            if tt in PSUM_TTS:
                return
            t = vsb.tile([PT, HS], bf16, tag="vsb", bufs=N * (TT - len(PSUM_TTS)),
                         name=f"vsb{tt}_{n}")
            nc.scalar.copy(t[:], v_ps[tt, n][:])
            v_sb[tt, n] = t

        # ---- combine + transpose + out-proj emission helpers ----
        h_tiles = {}

        def _emit_combine(tt):
            h = hp.tile([PT, HS], bf16, tag="h", bufs=4, name=f"h{tt}")
            gps_adds = tt in GPS_TTS and tt != TT - 1
            for n in range(N):
                src = v_ps[tt, n] if tt in PSUM_TTS else v_sb[tt, n]
                av_b = avv[:, tt, n, :, :].broadcast_to([PT, HD, HEADS])
                if n == 0:
                    tgt = h
                else:
                    ptag = f"prodG{tt}" if gps_adds else "prod"
                    pbufs = N if gps_adds else 3
                    tgt = prodp.tile([PT, HS], bf16, tag=ptag, bufs=pbufs,
                                     name=f"prod{tt}_{n}")
                nc.vector.tensor_mul(
                    tgt[:].rearrange("p (dd e) -> p dd e", e=HEADS),
                    src[:].rearrange("p (dd e) -> p dd e", e=HEADS),
                    av_b,
                )
                if n > 0:
                    eng = nc.gpsimd if gps_adds else nc.vector
                    eng.tensor_add(h[:], h[:], tgt[:])
            h_tiles[tt] = h

        ht_sb = {}
        tr_done = {}

        def _emit_transposes(tt):
            g = tt // GRP
            if tt % GRP == 0:
                ht_sb[g] = htp.tile([PT, KHS * GRP * PT], bf16, tag="ht",
                                    name=f"ht{g}")
            tr = psT.tile([PT, KHS * PT], bf16, tag="tr", name=f"tr{tt}")
            h = h_tiles[tt]
            for m in range(KHS):
                nc.tensor.transpose(tr[:, m * PT:(m + 1) * PT],
                                    h[:, m * PT:(m + 1) * PT], eye_sb[:])
            tr_done[tt] = tr

        def _emit_htcopy(tt):
            g = tt // GRP
            out_view = ht_sb[g][:].rearrange(
                "p (m q c) -> p m q c", m=KHS, q=GRP)[:, :, tt % GRP, :]
            nc.scalar.copy(
                out_view,
                tr_done[tt][:].rearrange("p (m c) -> p m c", m=KHS))

        def _emit_outproj(g):
            for m2 in range(CH // PT):
                o_ps = psO.tile([PT, GRP * PT], f32, tag="o",
                                name=f"ops{g}_{m2}")
                for k in range(KHS):
                    nc.tensor.matmul(
                        o_ps[:],
                        wo_sb[k][:, m2 * PT:(m2 + 1) * PT],
                        ht_sb[g][:, k * GRP * PT:(k + 1) * GRP * PT],
                        start=(k == 0), stop=(k == KHS - 1),
                    )
                o_sb = outp.tile([PT, GRP * PT], bf16, tag="osb",
                                 name=f"osb{g}_{m2}")
                nc.scalar.activation(o_sb[:], o_ps[:], AF.Identity,
                                     bias=bo_sb[m2][:])
                nc.sync.dma_start(
                    d["out"][m2 * PT:(m2 + 1) * PT,
                             g * GRP * PT:(g + 1) * GRP * PT],
                    o_sb[:])

        # ---- pass-1 emission schedule ----
        # PE: v(0..3) tr0 tr1 v(4) tr2 tr3 v(5) v(6) tr4 v(7) tr5 outg0
        #     tr6 tr7 outg1   (trN emitted once h[N] exists in program order)
        # DVE: combine(tt) in order (muls + its DVE adds)
        # ACT: v-copies in (tt, n) order, ht-copies lagged, biases at groups
        # Interleave by emitting in consumption order; the Tile framework
        # serializes per-engine by emission order and inserts semaphores.
        for n in range(N):
            _emit_v(0, n)
            _emit_vcopy(0, n)
        for n in range(N):
            _emit_v(1, n)
            _emit_vcopy(1, n)
        _emit_combine(0)
        for n in range(N):
            _emit_v(2, n)
            _emit_vcopy(2, n)
        _emit_combine(1)
        for n in range(N):
            _emit_v(3, n)
            _emit_vcopy(3, n)
        _emit_combine(2)
        _emit_transposes(0)
        _emit_transposes(1)
        for n in range(N):
            _emit_v(4, n)
            _emit_vcopy(4, n)
        _emit_combine(3)
        _emit_htcopy(0)
        _emit_htcopy(1)
        _emit_transposes(2)
        for n in range(N):
            _emit_v(5, n)
            _emit_vcopy(5, n)
        _emit_combine(4)
        _emit_htcopy(2)
        _emit_transposes(3)
        for n in range(N):
            _emit_v(6, n)
            _emit_vcopy(6, n)
        _emit_combine(5)
        _emit_htcopy(3)
        _emit_transposes(4)
        for n in range(N):
            _emit_v(7, n)
            _emit_vcopy(7, n)
        _emit_outproj(0)
        _emit_combine(6)
        _emit_htcopy(4)
        _emit_transposes(5)
        _emit_combine(7)
        _emit_htcopy(5)
        _emit_transposes(6)
        _emit_htcopy(6)
        _emit_transposes(7)
        _emit_htcopy(7)
        _emit_outproj(1)


def _build_nc():
    import concourse.tile as tile
    from concourse import bacc, mybir

    f32 = mybir.dt.float32
    bf16 = mybir.dt.bfloat16
    nc = bacc.Bacc(
        "TRN2",
        target_bir_lowering=False,
        debug=False,
        enable_asserts=False,
        num_devices=NCORES,
    )
    d = {
        "c": nc.dram_tensor("c", [N, CH, T], bf16, kind="ExternalInput").ap(),
        "wv": nc.dram_tensor("wv", [CH, HS], bf16, kind="ExternalInput").ap(),
        "wq": nc.dram_tensor("wq", [CH, HEADS], bf16,
                             kind="ExternalInput").ap(),
        "wo": nc.dram_tensor("wo", [HS, CH], bf16, kind="ExternalInput").ap(),
        "bo": nc.dram_tensor("bo", [CH, 1], f32, kind="ExternalInput").ap(),
        "invc": nc.dram_tensor("invc", [CH, 1], bf16,
                               kind="ExternalInput").ap(),
        "eye": nc.dram_tensor("eye", [PT, PT], bf16, kind="ExternalInput").ap(),
        "out": nc.dram_tensor("out", [CH, T], bf16, kind="ExternalOutput").ap(),
    }
    with tile.TileContext(nc) as tc:
        _kernel_body(nc, tc, d)
    nc.compile()
    return nc


_NC_CACHE = None


def _get_nc():
    global _NC_CACHE
    if _NC_CACHE is None:
        _NC_CACHE = _build_nc()
    return _NC_CACHE


def _make_in_maps(q, c, rms_w, emb, w1, b1, w2, b2, w_kv, w_out, b_out):
    q = np.asarray(q).astype(np.int64)
    c = np.asarray(c, dtype=np.float32)
    rms_w = np.asarray(rms_w, dtype=np.float32)
    emb = np.asarray(emb, dtype=np.float32)
    w1 = np.asarray(w1, dtype=np.float32)
    b1 = np.asarray(b1, dtype=np.float32)
    w2 = np.asarray(w2, dtype=np.float32)
    b2 = np.asarray(b2, dtype=np.float32)
    w_kv = np.asarray(w_kv, dtype=np.float32)
    w_out = np.asarray(w_out, dtype=np.float32)
    b_out = np.asarray(b_out, dtype=np.float32)

    # query path (tiny: 8 vectors), exact fp32 math as the reference
    qe = emb[q]                                   # [B, CH]
    x1 = qe @ w1 + b1
    h1 = x1 * (1.0 / (1.0 + np.exp(-x1)))         # silu
    qh = (h1 @ w2 + b2).reshape(B, HEADS, HD)

    wkv3 = w_kv.reshape(CH, HEADS, 2 * HD)
    w_k = wkv3[:, :, :HD]                         # [CH, HEADS, HD]
    w_v = wkv3[:, :, HD:]
    wv = (rms_w[:, None, None] * w_v)             # [CH, HEADS, HD]
    # (d, e) column order: col d*HEADS+e
    wv_de = np.ascontiguousarray(
        wv.transpose(0, 2, 1).reshape(CH, HS), dtype=np.float32)
    scale = float(HD) ** -0.5
    # wq[b, ch, e] = rms_w[ch] * scale * sum_d w_k[ch, e, d] * qh[b, e, d]
    wq_all = np.einsum("ced,bed->bce", w_k, qh).astype(np.float32)
    wq_all = wq_all * (scale * rms_w[None, :, None])

    # out proj rows reordered to (d, e): row d*HEADS+e was row e*HD+d
    wo_de = np.ascontiguousarray(
        w_out.reshape(HEADS, HD, CH).transpose(1, 0, 2).reshape(HS, CH),
        dtype=np.float32)

    import ml_dtypes
    bf = ml_dtypes.bfloat16
    shared = {
        "wv": wv_de.astype(bf),
        "wo": wo_de.astype(bf),
        "bo": np.ascontiguousarray(b_out.reshape(CH, 1), dtype=np.float32),
        "invc": np.full((CH, 1), 1.0 / CH, dtype=np.float32).astype(bf),
        "eye": np.eye(PT, dtype=np.float32).astype(bf),
    }
    in_maps = []
    for b in range(B):
        m = dict(shared)
        m["c"] = np.ascontiguousarray(c[b].reshape(N, CH, T)).astype(bf)
        m["wq"] = np.ascontiguousarray(wq_all[b]).astype(bf)
        in_maps.append(m)
    return in_maps


def _run(in_maps, **kwargs):
    from concourse import bass_utils

    nc = _get_nc()
    return bass_utils.run_bass_kernel_spmd(
        nc, in_maps, core_ids=list(range(NCORES)), **kwargs)


def kernel(q, c, rms_w, emb, w1, b1, w2, b2, w_kv, w_out, b_out):
    in_maps = _make_in_maps(q, c, rms_w, emb, w1, b1, w2, b2, w_kv, w_out,
                            b_out)
    res = _run(in_maps)
    outs = [np.asarray(res.results[b]["out"]).astype(np.float32)
            .reshape(CH, H, W) for b in range(B)]
    return np.stack(outs, axis=0)
